# revision 39
# baseline (speedup 1.0000x reference)
"""Trainium2 Bass kernel for nn_BiLSTM2D (8-core SPMD, no collectives).

Math (validated in numpy vs the jax reference to ~2e-7 rel):
  - gln with g=1,b=0 folds to xn = alpha*x + beta, alpha/beta per-batch scalars
    computed on device from x stats.
  - The unfold(win=8,stride=2) + conv1d(K=5,pad=2) input path collapses to a
    16-tap "composite" conv over the f axis (contraction 64c x 16j), realized
    as 8 matmuls over (j, j+8) shift pairs against an X copy whose upper 64
    partitions hold x shifted by +8.  4 boundary l-columns (l in {0,1,59,60})
    use dedicated composite-weight variants (conv zero-padding of the unfold
    axis is l-dependent there).
  - beta-terms fold to D[o, class(l)] = beta*S + b_ih + b_hh, injected into
    PSUM via a tiny indicator matmul; alpha folds into an alpha*I identity
    matmul that injects the precomputed input-gate tensor into PSUM.
  - The recurrent scan runs 32 steps over windows; each core owns 4 of the 32
    pseudo-batch rows (batch b = core//2, window-offsets p0..p0+3), fully
    independent -> zero inter-core communication.
  - ConvTranspose1d(K=8,stride=2) is 4 shifted matmuls with (co, f-parity)
    packed in the 128 output partitions; prelu(prelu(x)) = leaky(x, a^2) is
    realized as 0.9375*relu(z) + 0.0625*z with the bias folded in.
"""

import os
import sys
import types

import numpy as np
import ml_dtypes

BF16 = ml_dtypes.bfloat16
F8 = ml_dtypes.float8_e4m3

B, C, T, F = 4, 64, 256, 128
WIN, STRIDE, HID = 8, 2, 64
NWIN = T // WIN            # 32
L = (F - WIN) // STRIDE + 1  # 61
NPC = 4                    # pseudo-batch rows per core
NCORES = 8
NCOL = NWIN * NPC          # 128 (w-major, p inner)
NBLK = 16                  # column blocks of 8
CNT = float(C * T * F)     # gln element count per batch
VALID_DK = {0: [2, 3, 4], 1: [1, 2, 3, 4], 2: [0, 1, 2, 3, 4],
            3: [0, 1, 2, 3], 4: [0, 1, 2]}
BOUND_L = [(0, 0), (1, 1), (L - 2, 3), (L - 1, 4)]  # (l, variant)


def _cls_of_l(l):
    return {0: 0, 1: 1, L - 2: 3, L - 1: 4}.get(l, 2)


# ---------------------------------------------------------------- host packing

def _composite(W_ih):
    W = np.asarray(W_ih, np.float32).reshape(256, 64, 8, 5)  # [o, c, k, dk]
    out = {}
    for v, dks in VALID_DK.items():
        Wc = np.zeros((256, 64, 16), np.float32)
        for dk in dks:
            for k in range(8):
                Wc[:, :, 2 * dk + k] += W[:, :, k, dk]  # j+4 = 2dk+k
        out[v] = Wc
    return out


def _pack_host(inputs):
    x = np.asarray(inputs['x'], np.float32)
    Wf = np.asarray(inputs['W_ih_f'], np.float32)
    Wb = np.asarray(inputs['W_ih_b'], np.float32)
    bf = np.asarray(inputs['b_ih_f'], np.float32)
    bb = np.asarray(inputs['b_ih_b'], np.float32)
    Whf = np.asarray(inputs['W_hh_f'], np.float32)[:, :, 0]
    Whb = np.asarray(inputs['W_hh_b'], np.float32)[:, :, 0]
    bhf = np.asarray(inputs['b_hh_f'], np.float32)
    bhb = np.asarray(inputs['b_hh_b'], np.float32)
    Wp = np.asarray(inputs['W_proj'], np.float32)
    bp = np.asarray(inputs['b_proj'], np.float32)

    shared = {}
    # composite conv lhsT, split by gate group:
    #   fp8 DoubleRow groups (G-idx, dir, oc, scale): i=(0,0,0,1), f=(1,0,1,1),
    #   o=(3,1,1,0.5 tanh-trick fold); bf16 group: g=(2,1,0).
    # comp8: [128p, 5v, 3grp, 4qpair, 2ktile, 128f]; comp16: [128p, 5v, 8jp, 128f]
    Wcf, Wcb = _composite(Wf), _composite(Wb)
    comp8 = np.zeros((128, 5, 3, 4, 2, 128), np.float32)
    comp16 = np.zeros((128, 5, 8, 128), np.float32)
    F8GRP = [(0, 0, 1.0), (0, 1, 1.0), (1, 1, 1.0)]  # (d, oc, scale)
    for v in range(5):
        for gi, (d, oc, sc) in enumerate(F8GRP):
            Wc = (Wcf, Wcb)[d]
            for q in range(4):
                for k in range(2):
                    jp = 2 * q + k
                    comp8[0:64, v, gi, q, k, :] = sc * Wc[v][oc * 128:(oc + 1) * 128, :, jp].T
                    comp8[64:128, v, gi, q, k, :] = sc * Wc[v][oc * 128:(oc + 1) * 128, :, jp + 8].T
        for jp in range(8):
            comp16[0:64, v, jp, :] = Wcb[v][0:128, :, jp].T
            comp16[64:128, v, jp, :] = Wcb[v][0:128, :, jp + 8].T
    shared['comp8'] = comp8.astype(F8)
    shared['comp16'] = comp16.astype(BF16)

    # sigma-form gates: all four gates through one Sigmoid ACT.  The g-gate
    # (tanh) uses tanh(x) = 2*sigmoid(2x)-1, so every g-gate contribution is
    # DOUBLED (whh rows, spack/bpack rows; the conv part gets 2*alpha at the
    # G''-fold).  o-gate is plain sigmoid (no halving).
    whh = np.zeros((128, 4, 128), np.float32)
    whh[0:64, 0, :] = Whf[0:128].T
    whh[0:64, 1, :] = Whf[128:256].T
    whh[64:128, 2, :] = 2.0 * Whb[0:128].T
    whh[64:128, 3, :] = Whb[128:256].T
    shared['whh'] = whh.astype(BF16)

    shared['ident'] = np.eye(128, dtype=np.float32).astype(BF16)

    # l-class indicator for the D_exp build: [5cls, L]
    ind5 = np.zeros((5, L), np.float32)
    for l in range(L):
        ind5[_cls_of_l(l), l] = 1.0
    shared['ind5'] = ind5.astype(BF16)

    # S sums + biases: [5cls, 2gh, 2bank, 128]; grp = 2*bank + gh
    spack = np.zeros((5, 2, 2, 128), np.float32)
    bpack = np.zeros((5, 2, 2, 128), np.float32)
    for bank, (W, bi, bh) in enumerate(((Wf, bf, bhf), (Wb, bb, bhb))):
        for gh in range(2):
            osl = slice(gh * 128, (gh + 1) * 128)
            sc = 2.0 if (bank == 1 and gh == 0) else 1.0  # g-gate doubling
            for v in range(5):
                spack[v, gh, bank, :] = sc * W[osl][:, :, VALID_DK[v]].sum(axis=(1, 2))
                bpack[v, gh, bank, :] = sc * (bi[osl] + bh[osl])
    shared['spack'] = spack
    shared['bpack'] = bpack

    wproj = np.zeros((128, 4, 128), np.float32)
    for j in range(4):
        for r in range(2):
            wproj[:, j, r * 64:(r + 1) * 64] = Wp[:, :, r + 2 * j]
    shared['wproj'] = wproj.astype(BF16)

    bpp = np.concatenate([bp, bp]).reshape(128, 1)
    shared['bp9375'] = (0.9375 * bpp).astype(np.float32)

    in_maps = []
    for i in range(NCORES):
        b, p0 = i // 2, 4 * (i % 2)
        tf = (8 * np.arange(NWIN)[:, None] + (p0 + np.arange(NPC))[None, :]).reshape(-1)
        Xf = x[b][:, tf, :]            # [64, 128, 128]
        Xb = x[b][:, 255 - tf, :]
        m = {}
        for name, X in (('x2f', Xf), ('x2b', Xb)):
            x2 = np.zeros((128, NCOL, 128), np.float32)
            x2[0:64, :, 4:128] = X[:, :, 0:124]
            x2[64:128, :, 0:124] = X[:, :, 4:128]
            if name == 'x2b':
                m[name] = x2.astype(BF16)
            m[name + '8'] = x2.astype(F8)
        resid = np.empty((128, NCOL, 64), np.float32)
        resid[0:64] = Xf[:, :, 0::2]
        resid[64:128] = Xf[:, :, 1::2]
        resid += 0.0625 * bpp[:, :, None]  # fold the lt-branch bias in
        m['resid'] = resid
        m.update(shared)
        in_maps.append(m)
    return in_maps


# ---------------------------------------------------------------- device build

def _build():
    import concourse.bacc as bacc
    import concourse.mybir as mybir
    import concourse.tile as tile
    from concourse.ap import AP

    dt = mybir.dt
    AF = mybir.ActivationFunctionType
    ALU = mybir.AluOpType
    PM = mybir.MatmulPerfMode
    nc = bacc.Bacc("TRN2", target_bir_lowering=False, debug=False,
                   num_devices=NCORES)

    def din(name, shape, dty=dt.bfloat16):
        return nc.dram_tensor(name, shape, dty, kind="ExternalInput").ap()

    x2b_d = din('x2b', [128, NCOL, 128])
    x2f8_d = din('x2f8', [128, NCOL, 128], dt.float8e4)
    x2b8_d = din('x2b8', [128, NCOL, 128], dt.float8e4)
    resid_d = din('resid', [128, NCOL, 64], dt.float32)
    comp8_d = din('comp8', [128, 5, 3, 4, 2, 128], dt.float8e4)
    comp16_d = din('comp16', [128, 5, 8, 128])
    whh_d = din('whh', [128, 4, 128])
    ident_d = din('ident', [128, 128])
    ind5_d = din('ind5', [5, L])
    spack_d = din('spack', [5, 2, 2, 128], dt.float32)
    bpack_d = din('bpack', [5, 2, 2, 128], dt.float32)
    wproj_d = din('wproj', [128, 4, 128])
    bp9375_d = din('bp9375', [128, 1], dt.float32)
    y_d = nc.dram_tensor('y', [128, NCOL, 64], dt.float32, kind="ExternalOutput").ap()

    LSL = (slice(0, 31), slice(31, L))  # phase-2 l-streams

    with tile.TileContext(nc) as tc:
        with tc.tile_pool(name="persist", bufs=1) as P, \
             tc.tile_pool(name="ph2ps", bufs=2, space="PSUM") as P2, \
             tc.tile_pool(name="ph1ps", bufs=2, space="PSUM") as PP, \
             tc.tile_pool(name="ph1bs", bufs=1, space="PSUM") as PBo, \
             tc.tile_pool(name="p3x", bufs=1, space="PSUM") as P3, \
             tc.tile_pool(name="wbpool", bufs=1) as WB, \
             tc.tile_pool(name="ph3s", bufs=1) as S3, \
             tc.tile_pool(name="ph2s", bufs=2) as S2:

            # ---- persistent SBUF tiles
            X2b = P.tile([128, NCOL, 128], dt.bfloat16)
            X2f8 = P.tile([128, NCOL, 128], dt.float8e4)
            X2b8 = P.tile([128, NCOL, 128], dt.float8e4)
            WtI8 = P.tile([128, 3, 4, 2, 128], dt.float8e4)  # interior i/f/o DR
            WtI16 = P.tile([128, 8, 128], dt.bfloat16)       # interior g-gate
            WhhT = P.tile([128, 4, 128], dt.bfloat16)
            IdT = P.tile([128, 128], dt.bfloat16)
            Ind5 = P.tile([5, L], dt.bfloat16)
            SpT = P.tile([5, 2, 2, 128], dt.float32)
            BpT = P.tile([5, 2, 2, 128], dt.float32)
            WpT = P.tile([128, 4, 128], dt.bfloat16)
            Bp9 = P.tile([128, 1], dt.float32)
            G = P.tile([128, 4, NWIN, NPC, L], dt.bfloat16)
            HH = P.tile([128, NWIN, NPC, 67], dt.bfloat16)
            Ct = P.tile([128, NPC, L], dt.bfloat16)
            BNS = P.tile([64, 2, 32, 6], dt.float32)   # bn_stats 6-tuples (low)
            ACS = P.tile([128, 4], dt.float32)         # high-partition accums
            SCRH = P.tile([64, NCOL, 4], dt.bfloat16)  # accum dummy out
            MV = P.tile([64, 2], dt.float32)           # bn_aggr (mean, var)
            SXX = P.tile([128, 2], dt.float32)         # per-partition Sx, Sxx
            STL = P.tile([1, 12], dt.float32)
            ONES128 = P.tile([128, 1], dt.float32)
            ONES1 = P.tile([1, 128], dt.float32)
            AB = P.tile([128, 2], dt.float32)
            AB2 = P.tile([128, 1], dt.float32)
            Dt = P.tile([5, 2, 2, 128], dt.bfloat16)
            AlphaI = P.tile([128, 128], dt.bfloat16)
            AB3 = P.tile([128, 1], dt.float32)         # sigma = 1/alpha
            Dx = P.tile([128, 4, NPC, L], dt.bfloat16)  # D/alpha, NPC-replicated

            # ---- input DMAs (X2 chunked so phase-1 matmuls start early;
            # spread across engine DGE queues so transfers run in parallel)
            nc.sync.dma_start(WtI8[:], comp8_d[:, 2])       # interior weights first
            nc.sync.dma_start(WtI16[:], comp16_d[:, 2])
            for ch in range(4):
                csl = slice(32 * ch, 32 * (ch + 1))
                nc.sync.dma_start(X2f8[:, csl], x2f8_d[:, csl])
                nc.gpsimd.dma_start(X2b8[:, csl], x2b8_d[:, csl])
                nc.scalar.dma_start(X2b[:, csl], x2b_d[:, csl])
            nc.sync.dma_start(WhhT[:], whh_d[:])
            nc.sync.dma_start(IdT[:], ident_d[:])
            nc.sync.dma_start(Ind5[:], ind5_d[:])
            nc.sync.dma_start(SpT[:], spack_d[:])
            nc.sync.dma_start(BpT[:], bpack_d[:])
            nc.sync.dma_start(WpT[:], wproj_d[:])
            nc.sync.dma_start(Bp9[:], bp9375_d[:])

            # only the l-padding columns of HH must be zero (for phase 3)
            nc.gpsimd.memset(HH[:, :, :, 0:3], 0.0)
            nc.gpsimd.memset(HH[:, :, :, 64:67], 0.0)
            nc.vector.memset(ACS[:], 0.0)
            nc.vector.memset(ONES128[:], 1.0)
            nc.vector.memset(ONES1[:], 1.0)

            # ---- gln stats: bn_stats over 512-elem contiguous chunks for the
            # low partitions (zero-pad columns included: they shift count/mean
            # but not the sums, which is all we extract).  The 4 leftover
            # f-columns live on the high partitions -> plain accum sums.
            def flat512(X2, c):
                base = X2[0:64, 4 * c:4 * c + 4, :]
                p = base.ap.to_list()
                return AP(base.tensor, base.offset, [[p[0][0], 64], [1, 512]])

            def stats_chunk(ch):
                for d, X2 in enumerate((X2f8, X2b)):
                    for c in range(8 * ch, 8 * ch + 8):
                        nc.vector.bn_stats(BNS[:, d, c, :], flat512(X2, c))
                    slu = X2[64:128, :, 120:124]
                    if ch == 3:  # accum passes need full NCOL; emit on last
                        nc.vector.tensor_scalar(
                            SCRH[:], slu, 1.0, 0.0, op0=ALU.mult, op1=ALU.add,
                            accum_out=ACS[64:128, d:d + 1])
                        nc.vector.scalar_tensor_tensor(
                            SCRH[:], slu, 1.0, slu, op0=ALU.mult, op1=ALU.mult,
                            accum_out=ACS[64:128, 2 + d:3 + d])

            def stats_finish():
                NBN = 2.0 * 32.0 * 512.0  # bn sample count per low partition
                nc.vector.bn_aggr(MV[:], BNS[:])
                nc.vector.tensor_scalar_mul(SXX[0:64, 0:1], MV[:, 0:1], NBN)
                nc.vector.tensor_mul(SXX[0:64, 1:2], MV[:, 0:1], MV[:, 0:1])
                nc.vector.tensor_add(SXX[0:64, 1:2], SXX[0:64, 1:2], MV[:, 1:2])
                nc.vector.tensor_scalar_mul(SXX[0:64, 1:2], SXX[0:64, 1:2], NBN)
                nc.vector.tensor_add(SXX[64:128, 0:1], ACS[64:128, 0:1],
                                     ACS[64:128, 1:2])
                nc.vector.tensor_add(SXX[64:128, 1:2], ACS[64:128, 2:3],
                                     ACS[64:128, 3:4])
                ps_s = P3.tile([1, 2], dt.float32, tag="p3x")
                nc.tensor.matmul(ps_s[:], ONES128[:], SXX[:],
                                 start=True, stop=True)
                nc.vector.tensor_scalar_mul(STL[0:1, 2:3], ps_s[0:1, 0:1], 1.0 / CNT)
                nc.vector.tensor_scalar_mul(STL[0:1, 3:4], ps_s[0:1, 1:2], 1.0 / CNT)
                nc.vector.tensor_mul(STL[0:1, 4:5], STL[0:1, 2:3], STL[0:1, 2:3])
                nc.vector.tensor_sub(STL[0:1, 5:6], STL[0:1, 3:4], STL[0:1, 4:5])
                nc.vector.tensor_scalar_add(STL[0:1, 6:7], STL[0:1, 5:6], 1e-8)
                nc.scalar.sqrt(STL[0:1, 7:8], STL[0:1, 6:7])
                nc.vector.reciprocal(STL[0:1, 8:9], STL[0:1, 7:8])     # alpha
                nc.vector.tensor_mul(STL[0:1, 9:10], STL[0:1, 2:3], STL[0:1, 8:9])
                nc.vector.tensor_scalar_mul(STL[0:1, 10:11], STL[0:1, 9:10], -1.0)
                ps_ab = P3.tile([128, 24], dt.float32, tag="p3x")
                nc.tensor.matmul(ps_ab[:, 0:2], ONES1[:], STL[0:1, 8:11:2],
                                 start=True, stop=True)
                nc.tensor.matmul(ps_ab[:, 2:3], ONES1[:], STL[0:1, 7:8],
                                 start=True, stop=True)
                nc.vector.tensor_copy(AB[:], ps_ab[:, 0:2])
                nc.vector.tensor_copy(AB3[:], ps_ab[:, 2:3])
                nc.vector.tensor_scalar_mul(AlphaI[:], IdT[:], AB[:, 0:1])
                nc.vector.scalar_tensor_tensor(
                    Dt[:], SpT[:], AB[0:5, 1:2], BpT[:],
                    op0=ALU.mult, op1=ALU.add)
                psD = P3.tile([128, 4, L], dt.float32, tag="p3x")
                for bank in range(2):
                    for gh in range(2):
                        nc.tensor.matmul(psD[:, 2 * bank + gh], Dt[:, gh, bank, :],
                                         Ind5[:], start=True, stop=True)
                for p_ in range(NPC):
                    nc.vector.tensor_scalar_mul(Dx[:, :, p_, :], psD[:], AB3[:])

            # ---- G''-fold: G += D/alpha (the alpha scale rides the AlphaI
            # injection; the g-gate 2x is folded into its evac scale).
            # Even w on GpSimd, odd w on DVE.
            def fold_block(blk):
                for w in (2 * blk, 2 * blk + 1):
                    eng = nc.gpsimd if w % 2 == 0 else nc.vector
                    eng.tensor_add(G[:, :, w], G[:, :, w], Dx[:])

            # ---- DoubleRow rhs helpers: overlapped strided views on fp8 X2
            def dr_rhs_main(X8, blk, q):
                base = X8[:, 8 * blk:8 * blk + 8, 2 * q:2 * q + 122]
                p = base.ap.to_list()
                return AP(base.tensor, base.offset,
                          [[p[0][0], 128], [1, 2], [p[1][0], 8], [2, 61]])

            def dr_rhs_bound(X8, lb, q):
                base = X8[:, :, 2 * lb + 2 * q:2 * lb + 2 * q + 2]
                p = base.ap.to_list()
                return AP(base.tensor, base.offset,
                          [[p[0][0], 128], [1, 2], [p[1][0], NCOL]])

            F8G = [(0, X2f8), (1, X2f8), (3, X2b8)]  # (G grp, fp8 src) i,f,o

            # ---- phase 1 main-block emitter: one (w-pair) column block;
            #      i/f/o groups via fp8 DoubleRow, g-gate via bf16; evac on
            #      ScalarE (scales folded into weights host-side).
            def main_block(blk):
                cs = slice(8 * blk, 8 * blk + 8)
                for gi, (g, X8) in enumerate(F8G):
                    ps = PP.tile([128, 2, NPC, L], dt.float32, tag="ph1")
                    for q in range(4):
                        nc.tensor.matmul(ps[:], WtI8[:, gi, q],
                                         dr_rhs_main(X8, blk, q),
                                         start=(q == 0), stop=(q == 3),
                                         perf_mode=PM.DoubleRow)
                    nc.scalar.activation(
                        G[:, g, 2 * blk:2 * blk + 2, :, 2:L - 2],
                        ps[:, :, :, 2:L - 2], AF.Copy)
                ps = PP.tile([128, 2, NPC, L], dt.float32, tag="ph1")
                for jp in range(8):
                    nc.tensor.matmul(ps[:], WtI16[:, jp, :],
                                     X2b[:, cs, jp:jp + 121:2],
                                     start=(jp == 0), stop=(jp == 7))
                nc.scalar.activation(
                    G[:, 2, 2 * blk:2 * blk + 2, :, 2:L - 2],
                    ps[:, :, :, 2:L - 2], AF.Copy, scale=2.0)

            # boundary l-columns: dedicated composite weights, full width
            def boundary_all():
                for (lb, v) in BOUND_L:
                    WtB8 = WB.tile([128, 3, 4, 2, 128], dt.float8e4, tag="wb8")
                    WtB16 = WB.tile([128, 8, 128], dt.bfloat16, tag="wb16")
                    nc.sync.dma_start(WtB8[:], comp8_d[:, v])
                    nc.sync.dma_start(WtB16[:], comp16_d[:, v])
                    for gi, (g, X8) in enumerate(F8G):
                        psb = PBo.tile([128, NWIN, NPC], dt.float32, tag="ph1b")
                        for q in range(4):
                            nc.tensor.matmul(psb[:], WtB8[:, gi, q],
                                             dr_rhs_bound(X8, lb, q),
                                             start=(q == 0), stop=(q == 3),
                                             perf_mode=PM.DoubleRow)
                        nc.scalar.activation(G[:, g, :, :, lb], psb[:], AF.Copy)
                    psb = PBo.tile([128, NWIN, NPC], dt.float32, tag="ph1b")
                    for jp in range(8):
                        nc.tensor.matmul(psb[:], WtB16[:, jp, :],
                                         X2b[:, :, 2 * lb + jp],
                                         start=(jp == 0), stop=(jp == 7))
                    nc.scalar.activation(G[:, 2, :, :, lb], psb[:], AF.Copy,
                                         scale=2.0)

            # ---- phase 2 step, sigma-form, one independent l-stream:
            #      psum bk[128, 4grp, NPC, ln] = Id@G'' (+ Whh@h_prev), then
            #      ONE Sigmoid ACT covers all four gates (g doubled ->
            #      tanh(x) = 2*sig(2x)-1; o plain sigmoid).
            def ph2_step(w):
                bks, SAs, TCs = [], [], []
                for s in (0, 1):
                    ls = LSL[s]
                    ln = ls.stop - ls.start
                    lhh = slice(3 + ls.start, 3 + ls.stop)
                    bk = P2.tile([128, 4, NPC, ln], dt.float32, tag=f"bk{s}")
                    nc.tensor.matmul(bk[:], AlphaI[:], G[:, :, w, :, ls],
                                     start=True, stop=(w == 0))
                    if w > 0:
                        hprev = HH[:, w - 1, :, lhh]
                        for g in range(4):
                            nc.tensor.matmul(bk[:, g], WhhT[:, g], hprev,
                                             start=False, stop=(g == 3))
                    bks.append(bk)
                for s in (0, 1):
                    SA = S2.tile([128, 4, NPC, LSL[s].stop - LSL[s].start],
                                 dt.bfloat16, tag=f"sa{s}")
                    nc.scalar.activation(SA[:], bks[s][:], AF.Sigmoid)
                    SAs.append(SA)
                for s in (0, 1):
                    ls = LSL[s]
                    ln = ls.stop - ls.start
                    SA = SAs[s]
                    M2 = S2.tile([128, NPC, ln], dt.bfloat16, tag=f"m2{s}")
                    cv = Ct[:, :, ls]
                    # m2 = (sig(2g) - 0.5) * sig(i)  [= tanh(g)*sig(i)/2]
                    nc.vector.scalar_tensor_tensor(
                        M2[:], SA[:, 2], -0.5, SA[:, 0], op0=ALU.add,
                        op1=ALU.mult)
                    if w == 0:
                        nc.vector.tensor_scalar_mul(cv, M2[:], 2.0)
                    else:
                        nc.vector.tensor_mul(cv, cv, SA[:, 1])
                        nc.vector.scalar_tensor_tensor(
                            cv, M2[:], 2.0, cv, op0=ALU.mult, op1=ALU.add)
                for s in (0, 1):
                    ls = LSL[s]
                    TC = S2.tile([128, NPC, ls.stop - ls.start], dt.bfloat16,
                                 tag=f"tc{s}")
                    nc.scalar.activation(TC[:], Ct[:, :, ls], AF.Tanh)
                    TCs.append(TC)
                for s in (0, 1):
                    ls = LSL[s]
                    lhh = slice(3 + ls.start, 3 + ls.stop)
                    nc.gpsimd.tensor_mul(HH[:, w, :, lhh], SAs[s][:, 3], TCs[s][:])

            # ---- phase 3 block: conv-transpose + double-prelu + residual.
            #      0.0625*bp is pre-folded into resid on host.
            def ph3_block(blk):
                ps3 = P3.tile([128, 2, NPC, 64], dt.float32, tag="p3x")
                ws = slice(2 * blk, 2 * blk + 2)
                for j in range(4):
                    nc.tensor.matmul(ps3[:], WpT[:, j, :],
                                     HH[:, ws, :, 3 - j:67 - j],
                                     start=(j == 0), stop=(j == 3))
                rt = S3.tile([128, 2, NPC, 64], dt.bfloat16, tag="rt")
                lt = S3.tile([128, 2, NPC, 64], dt.bfloat16, tag="lt")
                rs = S3.tile([128, 2, NPC, 64], dt.float32, tag="rs")
                cs = slice(8 * blk, 8 * blk + 8)
                nc.sync.dma_start(rs[:], resid_d[:, cs])
                nc.scalar.activation(rt[:], ps3[:], AF.Relu,
                                     bias=Bp9[:], scale=0.9375)
                nc.vector.scalar_tensor_tensor(
                    lt[:], ps3[:], 0.0625, rt[:], op0=ALU.mult, op1=ALU.add)
                nc.vector.tensor_add(rs[:], rs[:], lt[:])
                nc.sync.dma_start(y_d[:, cs], rs[:])

            # ---- emission schedule: solid phase-1 (tensor-bound) with stats
            # on DVE behind it; folds ride with their blocks; a single scan
            # step weaves after each late block (chain resolves inside the
            # next block's matmul stream), then the drain runs with phase-3
            # interleaved.
            w_done, p3_done = 0, 0

            def drain_ph2(w_target):
                nonlocal w_done, p3_done
                while w_done < w_target:
                    ph2_step(w_done)
                    w_done += 1
                    if w_done % 2 == 0 and p3_done < w_done // 2 - 1:
                        ph3_block(p3_done)
                        p3_done += 1

            for blk in range(3):
                main_block(blk)
                stats_chunk(blk)
            boundary_all()
            for blk in range(3, 7):
                main_block(blk)
                if blk == 3:
                    stats_chunk(3)
            stats_finish()
            for blk in range(7):
                fold_block(blk)
            for blk in range(7, NBLK):
                main_block(blk)
                fold_block(blk)
                if blk >= 8:
                    drain_ph2(2 * (blk - 7))
            drain_ph2(NWIN)
            while p3_done < NBLK:
                ph3_block(p3_done)
                p3_done += 1

    nc.compile()
    return nc


_CACHED = None


def _get_program():
    global _CACHED
    if _CACHED is None:
        _CACHED = _build()
    return _CACHED


LAST_RESULT = None


def kernel(**inputs):
    global LAST_RESULT
    from concourse.bass_utils import run_bass_kernel_spmd

    # optional NTFF profiling shim (used when BASS_TRACE=1): register the
    # antenv.axon_hooks module the image lacks.
    if os.environ.get("BASS_TRACE") and 'antenv.axon_hooks' not in sys.modules:
        try:
            import trn_agent_boot.trn_boot as _tb
            _m = types.ModuleType('antenv.axon_hooks')
            _hook = _tb._ntff_profile_via_ctypes('/opt/axon/libaxon_pjrt.so')
            _m.get_axon_ntff_profile_hook = lambda: _hook
            sys.modules['antenv.axon_hooks'] = _m
        except Exception:
            pass

    nc = _get_program()
    in_maps = _pack_host(inputs)
    res = run_bass_kernel_spmd(nc, in_maps, list(range(NCORES)))
    LAST_RESULT = res

    out = np.empty((B, C, T, F), np.float32)
    for i in range(NCORES):
        b, p0 = i // 2, 4 * (i % 2)
        r_ = res.results[i]['y'].reshape(2, 64, NWIN, NPC, 64)
        tmp = r_.transpose(1, 2, 3, 4, 0).reshape(64, NCOL, 128)
        tcols = (8 * np.arange(NWIN)[:, None]
                 + (p0 + np.arange(NPC))[None, :]).reshape(-1)
        out[b][:, tcols, :] = tmp
    return out

